# revision 1
# baseline (speedup 1.0000x reference)
"""Trainium2 Bass kernel for nn_CrossAttentionFusionBlock.

Decomposition (validated vs reference in numpy, exact in fp32):
  q = dw3x3(q_w2) . conv1x1(q_w1) applied to x   == sum_t Gq_t @ x_shift_t
  k = g3x3(kv_w2[:96]) . conv1x1(kv_w1[:96]) (z) == sum_t Gk_t @ z_shift_t
  v = g3x3(kv_w2[96:]) . conv1x1(kv_w1[96:]) (z) == sum_t Gv_t @ z_shift_t
  S = q @ k^T (per-head 24x24 blocks used), nq/nk = row norms (Gram diagonals)
  attn = softmax(S / (nq nk) * temperature)  [host, tiny]
  out = (proj @ blockdiag(attn)) @ v         [pass B matmul]

Sharding: pure data parallel, sample b -> core b (8 samples, 8 cores).
"""

import numpy as np
import ml_dtypes

import bass_rust
import concourse.bass as bass
import concourse.tile as tile
from concourse import mybir
from concourse.bass_utils import run_bass_kernel_spmd

B, C, H, W = 8, 96, 256, 256
HEADS = 4
HW = H * W
WP = W + 2  # padded row stride in SBUF
R = 32  # image rows per slab
NSLAB = H // R
BF16 = mybir.dt.bfloat16
F32 = mybir.dt.float32

TAPS = [(dy, dx) for dy in (-1, 0, 1) for dx in (-1, 0, 1)]

# ---------------------------------------------------------------- walrus fix
# This walrus build allows only 1-2 sync-wait commands per instruction;
# Tile's final drain can carry more. Split excess waits onto extra Drains.
_MAX_WAITS = 1
_USE_NOP_SPLIT = True
_MAX_WAITS_BY_TYPE = {"InstDrain": 1, "InstNoOp": 1, "InstEventSemaphore": 1}


def _split_excess_waits(nc):
    for f in nc.m.functions:
        for b in f.blocks:
            insts = b.instructions
            i = 0
            while i < len(insts):
                inst = insts[i]
                si = getattr(inst, "sync_info", None)
                limit = _MAX_WAITS_BY_TYPE.get(type(inst).__name__, _MAX_WAITS)
                if si is not None and si.on_wait and len(si.on_wait) > limit:
                    waits = list(si.on_wait)
                    extra, keep = waits[:-limit], waits[-limit:]
                    inst.sync_info = bass_rust.SyncInfo(
                        on_wait=keep, on_update=list(si.on_update)
                    )
                    pre = []
                    for j, w in enumerate(extra):
                        cls = (
                            mybir.InstNoOp
                            if _USE_NOP_SPLIT
                            else mybir.InstDrain
                        )
                        nd = cls(name=f"{inst.name}-wsplit-{j}", engine=inst.engine)
                        nd.sync_info = bass_rust.SyncInfo(on_wait=[w], on_update=[])
                        pre.append(nd)
                    insts[i:i] = pre
                    i += len(pre)
                i += 1


# ---------------------------------------------------------------- host math
def _fold_weights(kv_w1, kv_w2, q_w1, q_w2):
    """Gq, Gk, Gv: [9, 96, 96] fp32 dense per-tap matrices."""
    W1 = kv_w1[:, :, 0, 0].astype(np.float64)  # [192, 96]
    Q1 = q_w1[:, :, 0, 0].astype(np.float64)  # [96, 96]
    Gq = np.zeros((9, C, C), np.float64)
    Gk = np.zeros((9, C, C), np.float64)
    Gv = np.zeros((9, C, C), np.float64)
    for t, (dy, dx) in enumerate(TAPS):
        Gq[t] = q_w2[:, 0, dy + 1, dx + 1].astype(np.float64)[:, None] * Q1
        for c in range(C):
            g2 = 2 * (c // 2)
            Gk[t, c] = (
                kv_w2[c, 0, dy + 1, dx + 1] * W1[g2]
                + kv_w2[c, 1, dy + 1, dx + 1] * W1[g2 + 1]
            )
            cc = C + c
            g2v = 2 * (cc // 2)
            Gv[t, c] = (
                kv_w2[cc, 0, dy + 1, dx + 1] * W1[g2v]
                + kv_w2[cc, 1, dy + 1, dx + 1] * W1[g2v + 1]
            )
    return Gq.astype(np.float32), Gk.astype(np.float32), Gv.astype(np.float32)


def _attn_fold(s_qk, s_kk, temperature, proj_w):
    """s_qk [96,192] = [q@qT | q@kT], s_kk [96,96] = k@kT.
    Returns M^T [96,96] fp32 (lhsT for pass B: out = M @ v)."""
    eps = 1e-12
    S = s_qk[:, C:].astype(np.float64)
    nq = np.maximum(np.sqrt(np.abs(np.diag(s_qk[:, :C]))), eps)
    nk = np.maximum(np.sqrt(np.abs(np.diag(s_kk))), eps)
    A = np.zeros((C, C), np.float64)
    hs = C // HEADS
    for h in range(HEADS):
        sl = slice(h * hs, (h + 1) * hs)
        logits = (S[sl, sl] / nq[sl, None] / nk[None, sl]) * float(
            temperature[h, 0, 0]
        )
        logits -= logits.max(axis=1, keepdims=True)
        e = np.exp(logits)
        A[sl, sl] = e / e.sum(axis=1, keepdims=True)
    M = proj_w[:, :, 0, 0].astype(np.float64) @ A
    return np.ascontiguousarray(M.T).astype(np.float32)


# ---------------------------------------------------------------- pass A
import os
_SKIP = set(os.environ.get("K_SKIP", "").split(","))


def _build_pass_a():
    nc = bass.Bass()
    x = nc.dram_tensor("x", [C, HW], F32, kind="ExternalInput")
    z = nc.dram_tensor("z", [C, HW], F32, kind="ExternalInput")
    gt = nc.dram_tensor("gt", [C, 27 * C], BF16, kind="ExternalInput")
    ident = nc.dram_tensor("ident", [128, 128], BF16, kind="ExternalInput")
    s_qk = nc.dram_tensor("s_qk", [C, 2 * C], F32, kind="ExternalOutput")
    s_kk = nc.dram_tensor("s_kk", [C, C], F32, kind="ExternalOutput")
    v_out = nc.dram_tensor("v_out", [C, HW], BF16, kind="ExternalOutput")

    x3 = x[:].rearrange("c (h w) -> c h w", w=W)
    z3 = z[:].rearrange("c (h w) -> c h w", w=W)
    v3 = v_out[:].rearrange("c (h w) -> c h w", w=W)

    with tile.TileContext(nc) as tc:
        with (
            tc.tile_pool(name="singles", bufs=1) as singles,
            tc.tile_pool(name="slabs", bufs=2) as slabs,
            tc.tile_pool(name="chunk", bufs=3) as chunk_pool,
            tc.tile_pool(name="qkt", bufs=4) as qkt_pool,
            tc.tile_pool(name="cpsum", bufs=2, space="PSUM") as cpsum,
            tc.tile_pool(name="spsum", bufs=1, space="PSUM") as spsum,
        ):
            gt_sb = singles.tile([C, 27 * C], BF16)
            nc.sync.dma_start(out=gt_sb[:], in_=gt[:])
            id_sb = singles.tile([128, 128], BF16)
            nc.sync.dma_start(out=id_sb[:], in_=ident[:])

            s_qk_ps = spsum.tile([C, 2 * C], F32, tag="s_qk")
            s_kk_ps = spsum.tile([C, C], F32, tag="s_kk")

            nblk = HW // 128  # total 128-px gram blocks
            blk = 0  # running gram block index

            for s in range(NSLAB):
                xs = slabs.tile([C, (R + 2) * WP], BF16, tag="xslab")
                zs = slabs.tile([C, (R + 2) * WP], BF16, tag="zslab")
                xs3 = xs.rearrange("c (r w) -> c r w", w=WP)
                zs3 = zs.rearrange("c (r w) -> c r w", w=WP)
                # zero the pad columns (0 and 257) of every local row
                for t_ in (xs3, zs3):
                    nc.vector.memset(t_[:, :, 0:1], 0.0)
                    nc.vector.memset(t_[:, :, WP - 1 : WP], 0.0)
                # rows: local r <-> global row s*R - 1 + r, r in [0, R+2)
                g0 = s * R - 1
                lo = max(0, -g0)  # first valid local row
                hi = min(R + 2, H - g0)  # one past last valid local row
                if lo > 0:
                    nc.vector.memset(xs3[:, 0:lo, :], 0.0)
                    nc.vector.memset(zs3[:, 0:lo, :], 0.0)
                if hi < R + 2:
                    nc.vector.memset(xs3[:, hi:, :], 0.0)
                    nc.vector.memset(zs3[:, hi:, :], 0.0)
                # cast loads (SWDGE): fp32 dram -> bf16 sbuf
                nc.gpsimd.dma_start(
                    out=xs3[:, lo:hi, 1 : W + 1], in_=x3[:, g0 + lo : g0 + hi, :]
                )
                nc.gpsimd.dma_start(
                    out=zs3[:, lo:hi, 1 : W + 1], in_=z3[:, g0 + lo : g0 + hi, :]
                )

                qsl = qkt_pool.tile([C, (R // 2) * 512], BF16, tag="qsl", bufs=2)
                ksl = qkt_pool.tile([C, (R // 2) * 512], BF16, tag="ksl", bufs=2)
                vsl = qkt_pool.tile([C, (R // 2) * 512], BF16, tag="vsl", bufs=2)
                for cch in range(R // 2):  # 2-row output chunks
                    q_ps = cpsum.tile([C, 512], F32, tag="q_ps")
                    k_ps = cpsum.tile([C, 512], F32, tag="k_ps")
                    v_ps = cpsum.tile([C, 512], F32, tag="v_ps")
                    r0 = 1 + 2 * cch  # local row of first output row
                    for t, (dy, dx) in enumerate(TAPS):
                        if "taps" in _SKIP:
                            break
                        rhs_x = xs3[:, r0 + dy : r0 + dy + 2, 1 + dx : 1 + dx + W]
                        rhs_z = zs3[:, r0 + dy : r0 + dy + 2, 1 + dx : 1 + dx + W]
                        st = t == 0
                        sp = t == 8
                        nc.tensor.matmul(
                            q_ps[:],
                            gt_sb[:, t * C : (t + 1) * C],
                            rhs_x,
                            start=st,
                            stop=sp,
                        )
                        nc.tensor.matmul(
                            k_ps[:],
                            gt_sb[:, (9 + t) * C : (10 + t) * C],
                            rhs_z,
                            start=st,
                            stop=sp,
                        )
                        nc.tensor.matmul(
                            v_ps[:],
                            gt_sb[:, (18 + t) * C : (19 + t) * C],
                            rhs_z,
                            start=st,
                            stop=sp,
                        )
                    nc.vector.tensor_copy(
                        qsl[:, 512 * cch : 512 * (cch + 1)], q_ps[:]
                    )
                    nc.scalar.copy(ksl[:, 512 * cch : 512 * (cch + 1)], k_ps[:])
                    nc.vector.tensor_copy(
                        vsl[:, 512 * cch : 512 * (cch + 1)], v_ps[:]
                    )

                nc.sync.dma_start(
                    out=v3[:, s * R : s * R + R, :], in_=vsl[:]
                )
                nblk_s = (R // 2) * 4  # 128-px blocks per slab
                qkt_sl = qkt_pool.tile(
                    [128, nblk_s, 2 * C], BF16, tag="qkt_sl", bufs=1
                )
                nc.sync.dma_start_transpose(qkt_sl[:, :, 0:C], qsl[:])
                nc.sync.dma_start_transpose(qkt_sl[:, :, C : 2 * C], ksl[:])
                for b4 in range(nblk_s):
                    st = blk == 0
                    sp = blk == nblk - 1
                    nc.tensor.matmul(
                        s_qk_ps[:],
                        qkt_sl[:, b4, 0:C],
                        qkt_sl[:, b4, :],
                        start=st,
                        stop=sp,
                    )
                    nc.tensor.matmul(
                        s_kk_ps[:],
                        qkt_sl[:, b4, C : 2 * C],
                        qkt_sl[:, b4, C : 2 * C],
                        start=st,
                        stop=sp,
                    )
                    blk += 1



            s_qk_sb = singles.tile([C, 2 * C], F32)
            s_kk_sb = singles.tile([C, C], F32)
            nc.vector.tensor_copy(s_qk_sb[:], s_qk_ps[:])
            nc.vector.tensor_copy(s_kk_sb[:], s_kk_ps[:])
            nc.sync.dma_start(out=s_qk[:], in_=s_qk_sb[:])
            nc.sync.dma_start(out=s_kk[:], in_=s_kk_sb[:])

    _split_excess_waits(nc)
    return nc


# ---------------------------------------------------------------- pass B
def _build_pass_b():
    nc = bass.Bass()
    v_in = nc.dram_tensor("v_in", [C, HW], BF16, kind="ExternalInput")
    mt = nc.dram_tensor("mt", [C, C], BF16, kind="ExternalInput")
    out = nc.dram_tensor("out", [C, HW], F32, kind="ExternalOutput")

    with tile.TileContext(nc) as tc:
        with (
            tc.tile_pool(name="singles", bufs=1) as singles,
            tc.tile_pool(name="vtiles", bufs=2) as vtiles,
            tc.tile_pool(name="otiles", bufs=2) as otiles,
            tc.tile_pool(name="psum", bufs=4, space="PSUM") as psum,
        ):
            mt_sb = singles.tile([C, C], BF16)
            nc.sync.dma_start(out=mt_sb[:], in_=mt[:])
            NCH = 8192
            for i in range(HW // NCH):
                v_sb = vtiles.tile([C, NCH], BF16, tag="v_sb")
                nc.sync.dma_start(
                    out=v_sb[:], in_=v_in[:, i * NCH : (i + 1) * NCH]
                )
                o_sb = otiles.tile([C, NCH], F32, tag="o_sb")
                for j in range(NCH // 512):
                    o_ps = psum.tile([C, 512], F32, tag="o_ps")
                    nc.tensor.matmul(
                        o_ps[:],
                        mt_sb[:],
                        v_sb[:, j * 512 : (j + 1) * 512],
                        start=True,
                        stop=True,
                    )
                    cp = nc.vector.tensor_copy if j % 2 == 0 else nc.scalar.copy
                    cp(o_sb[:, j * 512 : (j + 1) * 512], o_ps[:])
                nc.sync.dma_start(out=out[:, i * NCH : (i + 1) * NCH], in_=o_sb[:])

    _split_excess_waits(nc)
    return nc


# ---------------------------------------------------------------- driver
_NC_CACHE = {}


def _get_nc(name):
    if name not in _NC_CACHE:
        _NC_CACHE[name] = _build_pass_a() if name == "a" else _build_pass_b()
    return _NC_CACHE[name]


def kernel(x, z, kv_w1, kv_w2, q_w1, q_w2, proj_w, temperature, _profile=None):
    x = np.asarray(x, dtype=np.float32)
    z = np.asarray(z, dtype=np.float32)
    kv_w1 = np.asarray(kv_w1, dtype=np.float32)
    kv_w2 = np.asarray(kv_w2, dtype=np.float32)
    q_w1 = np.asarray(q_w1, dtype=np.float32)
    q_w2 = np.asarray(q_w2, dtype=np.float32)
    proj_w = np.asarray(proj_w, dtype=np.float32)
    temperature = np.asarray(temperature, dtype=np.float32)

    Gq, Gk, Gv = _fold_weights(kv_w1, kv_w2, q_w1, q_w2)
    # lhsT pack: [96(in), 27*96(out)]: taps 0-8 q, 9-17 k, 18-26 v
    G = np.concatenate([Gq, Gk, Gv], axis=0)  # [27, out, in]
    gt_pack = np.ascontiguousarray(
        G.transpose(2, 0, 1).reshape(C, 27 * C)
    ).astype(ml_dtypes.bfloat16)
    ident = np.eye(128, dtype=ml_dtypes.bfloat16)

    xf = x.reshape(B, C, HW)
    zf = z.reshape(B, C, HW)

    nc_a = _get_nc("a")
    in_maps_a = [
        {"x": xf[b], "z": zf[b], "gt": gt_pack, "ident": ident} for b in range(B)
    ]
    kw = dict(_profile) if _profile else {}
    res_a = run_bass_kernel_spmd(nc_a, in_maps_a, core_ids=list(range(B)), **kw)
    if _profile is not None:
        _profile["result_a"] = res_a

    mts = []
    for b in range(B):
        mt = _attn_fold(
            np.asarray(res_a.results[b]["s_qk"], np.float32),
            np.asarray(res_a.results[b]["s_kk"], np.float32),
            temperature,
            proj_w,
        )
        mts.append(mt.astype(ml_dtypes.bfloat16))

    nc_b = _get_nc("b")
    in_maps_b = [
        {"v_in": np.asarray(res_a.results[b]["v_out"]), "mt": mts[b]}
        for b in range(B)
    ]
    res_b = run_bass_kernel_spmd(nc_b, in_maps_b, core_ids=list(range(B)), **kw)
    if _profile is not None:
        _profile["result_b"] = res_b

    out = np.stack(
        [np.asarray(res_b.results[b]["out"], np.float32) for b in range(B)]
    )
    return out.reshape(B, C, H, W)

